# revision 51
# baseline (speedup 1.0000x reference)
"""Multi-head attention (B=2, T=2048, D=1024, H=16, Dh=64) on 8 TRN2 NeuronCores.

Sharding: core c = 4*b + g  ->  batch b in {0,1}, head-group g in {0..3}
(4 heads per core: data parallel on batch, tensor parallel on heads).
Each core computes, for its batch element and its 4 heads:

  Q.T/K.T = Wq/k_shard.T @ x.T + b      [256, 2048]  (head-dim on partitions)
  V'      = x @ Wv_interleaved + b      [2048, 260]  ([V_h | 1] per head)
  per head pair (2p, 2p+1), per 512-wide i-chunk:
    S.T   = K_h Q_h.T                   (two K=64 matmuls on disjoint PE
                                         row groups -> run concurrently)
    P.T   = exp(S.T / 8)                (no max-subtraction: |S|/8 <~ 6.6)
    acc   = [V_h | 1].T @ P.T           [65, 512]  row 64 = softmax denom
    attnT = acc[:64] * (1/acc[64])
  partial = attnT.T @ Wout_shard        [2048, 1024]

The partial sum over the 4 head groups plus b_out is done on the host
("all-reduce after out_proj"), as is the batch unshard.

The kernel is one flat software pipeline over the 8 attention groups.
The scalar engine's exp stream (128 x [128,1024] Exp ~ 147us) and the
PE matmul stream (~150us of issue time) are co-saturated; in steady
state the PE runs at 100% occupancy, so scheduling aims to keep both
engines from ever stalling each other:

  - input DMA uses one batched trigger per tensor (DMA triggers
    serialize at ~0.6us each on the sync queue) with x streamed in
    token-chunk-major order so the first scores issue ~13us in;
  - ~56 dummy matmuls warm the PE HAM clock gate (cold 1.2GHz ->
    2.4GHz needs ~3.4us of sustained activity) while the first DMAs
    land, and 16 more keep it warm across the final norm chain;
  - PV matmuls are emitted 3 exp-steps late (8 pe buffers) so a PV
    waiting on the shared PSUM accumulator never blocks a later score
    matmul at the head of the in-order PE queue;
  - softmax normalization first evacuates the accumulators to SBUF
    f16 (two [65,512] copies release the PSUM banks for the next
    group), then per head broadcasts the denominator row with a K=1
    matmul issued from partition 64 and takes a reciprocal of the
    broadcast tile; one [64,512] multiply per head writes attnT
    (head 2p+1 via tmp + SBUF->SBUF DMA, since DVE lanes cannot
    shift partitions);
  - V / K / Q projection chunks and out-projection tiles are spread
    across the exp steps as filler work sized to each group's slack;
  - the final group's out-projection reads tmp directly via split-K
    matmuls (no DMA wait on the tail), with PSUM evacuations
    alternating between the idle scalar engine and the vector engine;
  - output partials are written as f16 (the host sums them in f32).
"""

import os
from collections import defaultdict

import numpy as np

B, T, D = 2, 2048, 1024
H, DH = 16, 64
NCORES, GROUPS = 8, 4
HPC = H // GROUPS        # 4 heads per core
F = HPC * DH             # 256 features per core
FT = F // 128            # 2 feature tiles / head pairs
KTN = D // 128           # 8 contraction tiles
TT = T // 128            # 16 token tiles
NCH = 512                # matmul free-dim chunk
VW = DH + 1              # 65: V plus ones column
VF = HPC * VW            # 260: interleaved [V_h | 1] x 4 heads
NG = FT * (T // NCH)     # 8 attention groups
PVD = 3                  # PV emission delay in exp-steps

_prog = None
LAST_RESULT = None


def _build():
    from contextlib import ExitStack

    import concourse.mybir as mybir
    import concourse.tile as tile
    from concourse import bacc
    from concourse.bass import ts

    f32 = mybir.dt.float32
    f32r = mybir.dt.float32r
    f16 = mybir.dt.float16
    Exp = mybir.ActivationFunctionType.Exp

    nc = bacc.Bacc()
    xT = nc.dram_tensor("xT", [D, T], f16, kind="ExternalInput")
    wq = nc.dram_tensor("wq", [D, F], f16, kind="ExternalInput")
    wk = nc.dram_tensor("wk", [D, F], f16, kind="ExternalInput")
    # wv/bv come pre-interleaved from the host: column h*65+64 is a zero
    # weight column whose bias is 1.0, producing the [V_h | 1] layout that
    # supplies the softmax-denominator row of the PV matmul for free.
    wv = nc.dram_tensor("wv", [D, VF], f16, kind="ExternalInput")
    bq = nc.dram_tensor("bq", [F, 1], f32, kind="ExternalInput")
    bk = nc.dram_tensor("bk", [F, 1], f32, kind="ExternalInput")
    bv = nc.dram_tensor("bv", [1, VF], f16, kind="ExternalInput")
    wo = nc.dram_tensor("wo", [F, D], f16, kind="ExternalInput")
    # f16 partials: the host f32-sums 4 of them; quantization ~1e-4 << budget
    out = nc.dram_tensor("out", [T, D], f16, kind="ExternalOutput")

    with ExitStack() as ctx:
        tc = ctx.enter_context(tile.TileContext(nc))
        pers = ctx.enter_context(tc.tile_pool(name="pers", bufs=1))
        ptp = ctx.enter_context(tc.tile_pool(name="ptp", bufs=8))
        osb = ctx.enter_context(tc.tile_pool(name="osb", bufs=2))
        msc = ctx.enter_context(tc.tile_pool(name="msc", bufs=2))
        psq = ctx.enter_context(tc.tile_pool(name="psq", bufs=2, space="PSUM"))
        pss = ctx.enter_context(tc.tile_pool(name="pss", bufs=2, space="PSUM"))
        pso = ctx.enter_context(tc.tile_pool(name="pso", bufs=1, space="PSUM"))

        xt = pers.tile([128, KTN, T], f16, tag="xt")
        wqs = pers.tile([128, KTN, F], f16, tag="wqs")
        wks = pers.tile([128, KTN, F], f16, tag="wks")
        wvs = pers.tile([128, KTN, VF], f16, tag="wvs")
        bqc = pers.tile([128, FT, 1], f32, tag="bqc")
        bkc = pers.tile([128, FT, 1], f32, tag="bkc")
        bvr = pers.tile([1, VF], f16, tag="bvr")
        ones_f = pers.tile([1, 128], f32, tag="ones_f")
        ones16 = pers.tile([1, 128], f16, tag="ones16")
        ones2f = pers.tile([128, DH], f32, tag="ones2f")
        ones2d = pers.tile([128, DH], f16, tag="ones2d")
        wos = pers.tile([128, FT, D], f16, tag="wos")
        wosl = pers.tile([DH, FT, D], f16, tag="wosl")  # rows 64-127, base 0
        qt = pers.tile([128, FT, T], f16, tag="qt")
        kt = pers.tile([128, FT, T], f16, tag="kt")
        vs = pers.tile([128, TT, VF], f16, tag="vs")
        at = pers.tile([128, FT, T], f16, tag="at")

        # ISA memset can't target f16; memset f32 then copy-convert
        nc.vector.memset(ones_f[:], 1.0)
        nc.vector.tensor_copy(ones16[:], ones_f[:])
        nc.vector.memset(ones2f[:], 1.0)
        nc.vector.tensor_copy(ones2d[:], ones2f[:])

        # ~3.6us of dummy matmuls while the input DMA streams in: the PE
        # HAM clock gate needs ~3.4us of sustained activity to lift the
        # cold 4/8 throttle, so the real prologue projections run at
        # 2.4GHz instead of 1.2GHz
        warm = psq.tile([DH, DH], f32, tag="psq", name="warm")
        for _ in range(56):
            nc.tensor.matmul(warm[:], ones2d[:], ones2d[:],
                             start=True, stop=True)

        # ---- input DMA: one batched trigger per tensor/chunk (DMA trigger
        # instructions serialize at ~600ns each on the sync queue), ordered
        # for fastest pipeline start ----
        def kp(ap):          # [(k p) f] DRAM view -> [p k f]
            return ap.rearrange("(k p) f -> p k f", p=128)

        def xchunk(c):       # two half-K triggers -> two parallel queues
            h = KTN // 2
            nc.sync.dma_start(
                xt[:, 0:h, ts(c, NCH)],
                kp(xT[0:h * 128, ts(c, NCH)]))
            nc.sync.dma_start(
                xt[:, h:KTN, ts(c, NCH)],
                kp(xT[h * 128:D, ts(c, NCH)]))

        nc.sync.dma_start(wks[:, :, :], kp(wk[:, :]))
        xchunk(0)
        nc.sync.dma_start(wqs[:, :, :], kp(wq[:, :]))
        for ft in range(FT):
            nc.sync.dma_start(bkc[:, ft, :], bk[ts(ft, 128), :])
            nc.sync.dma_start(bqc[:, ft, :], bq[ts(ft, 128), :])
        nc.sync.dma_start(bvr[:], bv[:])
        nc.sync.dma_start(wvs[:, :, :], kp(wv[:, :]))
        xchunk(1)
        xchunk(2)
        xchunk(3)
        nc.sync.dma_start(wos[:, :, :], kp(wo[:, :]))  # out-proj weights last
        for ft in range(FT):
            nc.sync.dma_start(wosl[:, ft, :], wo[128 * ft + 64:128 * ft + 128, :])

        # ---- work units ----
        # qk chunks are split into two half-K thunks emitted one exp-step
        # apart so a projection never puts >1us of PE work between two
        # score matmuls in the in-order PE queue.
        def qk_chunk_parts(wsb, bcol, dst, ft, c):
            st = {}
            def go1():
                ps = psq.tile([128, NCH], f32, tag="psq", name="ps")
                st['ps'] = ps
                for k in range(KTN // 2):
                    nc.tensor.matmul(
                        ps[:],
                        wsb[:, k, ts(ft, 128)],
                        xt[:, k, ts(c, NCH)],
                        start=(k == 0), stop=False,
                    )
            def go2():
                ps = st['ps']
                for k in range(KTN // 2, KTN):
                    nc.tensor.matmul(
                        ps[:],
                        wsb[:, k, ts(ft, 128)],
                        xt[:, k, ts(c, NCH)],
                        start=False, stop=(k == KTN - 1),
                    )
                nc.vector.tensor_scalar_add(
                    dst[:, ft, ts(c, NCH)], ps[:], bcol[:, ft, :]
                )
            return go1, go2

        def v_tile(t):
            def go():
                pv = psq.tile([128, VF], f32, tag="psq", name="pv")
                for k in range(KTN):
                    nc.tensor.matmul(
                        pv[:], xt[:, k, ts(t, 128)], wvs[:, k, :],
                        start=(k == 0), stop=False,
                    )
                # bias via ones-row (also writes the denominator 1.0 cols)
                nc.tensor.matmul(
                    pv[:], ones16[:, 0:128], bvr[:], start=False, stop=True
                )
                nc.vector.tensor_copy(vs[:, t, :], pv[:])
            return go

        def outproj_tile(t):
            def go():
                ob = osb.tile([128, D], f16, tag="ob", name="ob")
                for c in range(D // NCH):
                    pp = psq.tile([128, NCH], f32, tag="psq", name="pp")
                    for ft in range(FT):
                        nc.tensor.matmul(
                            pp[:],
                            at[:, ft, ts(t, 128)],
                            wos[:, ft, ts(c, NCH)],
                            start=(ft == 0), stop=(ft == FT - 1),
                        )
                    nc.vector.tensor_copy(ob[:, ts(c, NCH)], pp[:])
                nc.sync.dma_start(out[ts(t, 128), :], ob[:])
            return go

        def make_scores(p, ic):
            def scores(j):
                # disjoint PE row groups (partitions 0-63 / 64-127): the two
                # K=64 matmuls execute concurrently
                sc = pss.tile([128, 2 * NCH], f32, tag="sc", name="sc")
                for hh in range(2):
                    nc.tensor.matmul(
                        sc[:, ts(hh, NCH)],
                        kt[hh * 64: hh * 64 + DH, p, ts(j, 128)],
                        qt[hh * 64: hh * 64 + DH, p, ts(ic, NCH)],
                        start=True, stop=True,
                    )
                return sc
            return scores

        seq = [(p, ic) for p in range(FT) for ic in range(T // NCH)]
        scores_of = {g: make_scores(*g) for g in seq}

        pe_tiles = {}      # global exp-step -> pe tile
        acc_of = {}        # gi -> (acc0, acc1)

        def pv_step(gi, j):
            p, _ = seq[gi]
            def go():
                if j == 0:
                    a0 = pso.tile([VW, NCH], f32, tag="acc0", name="acc0")
                    a1 = pso.tile([VW, NCH], f32, tag="acc1", name="acc1")
                    acc_of[gi] = (a0, a1)
                accs = acc_of[gi]
                pe = pe_tiles.pop(gi * TT + j)
                for hh in range(2):
                    nc.tensor.matmul(
                        accs[hh][:, :],
                        vs[:, j, (2 * p + hh) * VW: (2 * p + hh + 1) * VW],
                        pe[:, ts(hh, NCH)],
                        start=(j == 0), stop=(j == TT - 1),
                    )
            return go

        # softmax normalization for group gi, in three thunks:
        #   a) evacuate acc0/acc1 to SBUF f16 ([65,512] copies) -- this
        #      alone releases the accumulator PSUM banks for the next group
        #   b) per head: broadcast the denominator row across 64 partitions
        #      with a K=1 fp16 matmul issued from partition 64, then take
        #      the reciprocal of the broadcast PSUM tile
        #   c) attnT = num * (1/den): head 2p to at[0:64]; head 2p+1 via a
        #      [64,512] tmp + SBUF->SBUF DMA (DVE lanes can't shift
        #      partitions)
        def norm_parts(gi, tail=False):
            p, ic = seq[gi]
            st = {}
            def evac():
                acc0, acc1 = acc_of[gi]
                n0 = msc.tile([VW, NCH], f16, tag="n0", name="n0")
                n1 = msc.tile([VW, NCH], f16, tag="n1", name="n1")
                if tail:     # scalar engine is idle after the exp stream
                    nc.scalar.copy(n0[:], acc0[:])
                else:
                    nc.vector.tensor_copy(n0[:], acc0[:])
                nc.vector.tensor_copy(n1[:], acc1[:])
                st['n0'], st['n1'] = n0, n1
            def recip():
                for hh in range(2):
                    pbd = psq.tile([DH, NCH], f32, tag="psq", name="pbd")
                    nc.tensor.matmul(
                        pbd[:], ones2d[64:65, :],
                        st['n%d' % hh][DH:VW, :], start=True, stop=True,
                    )
                    rcb = msc.tile([DH, NCH], f32, tag="rcb%d" % hh,
                                   name="rcb")
                    nc.vector.reciprocal_approx_fast(rcb[:], pbd[:])
                    st['rcb%d' % hh] = rcb
            def muls():
                nc.vector.tensor_mul(
                    at[0:DH, p, ts(ic, NCH)], st['n0'][0:DH, :], st['rcb0'][:]
                )
                tmp = msc.tile([DH, NCH], f16, tag="tmp", name="tmp")
                nc.vector.tensor_mul(tmp[:], st['n1'][0:DH, :], st['rcb1'][:])
                if tail:
                    # final group: out-proj reads tmp directly (split-K
                    # matmuls) so no DMA round-trip sits on the tail
                    st['tmp'] = tmp
                else:
                    nc.sync.dma_start(at[64:128, p, ts(ic, NCH)], tmp[:])
            return evac, recip, muls, st

        # ---- static filler schedule (global exp-step -> thunks) ----
        GS = NG * TT
        work = defaultdict(list)
        epilogue = []
        def add(s, th):
            if s < GS:
                work[s].append(th)
            else:
                epilogue.append(th)

        def add_qk(s, *args):
            g1, g2 = qk_chunk_parts(*args)
            add(s, g1)
            add(s + 1, g2)

        for j in range(2, TT):                    # V proj just-in-time
            add(j - 1, v_tile(j))
        add(0, v_tile(0))
        add(0, v_tile(1))
        add_qk(0, wks, bkc, kt, 0, 1)             # K chunks for group 0
        add_qk(4, wks, bkc, kt, 0, 2)
        add_qk(8, wks, bkc, kt, 0, 3)
        add_qk(17, wks, bkc, kt, 1, 0)            # K chunks for groups 4-7,
        add_qk(21, wks, bkc, kt, 1, 1)            # spread over groups 1-3
        add_qk(33, wks, bkc, kt, 1, 2)
        add_qk(37, wks, bkc, kt, 1, 3)
        add_qk(12, wqs, bqc, qt, 0, 1)            # Q chunks, one group ahead
        add_qk(26, wqs, bqc, qt, 0, 2)
        add_qk(42, wqs, bqc, qt, 0, 3)
        add_qk(56, wqs, bqc, qt, 1, 0)
        add_qk(72, wqs, bqc, qt, 1, 1)
        add_qk(60, wqs, bqc, qt, 1, 2)            # keep the out-proj groups
        add_qk(76, wqs, bqc, qt, 1, 3)            # (5-7) free of projections
        for ic in range(3):                       # out-proj once both head
            for i, off in enumerate((5, 7, 9, 11)):    # pairs' attnT exist
                add((5 + ic) * TT + off, outproj_tile(4 * ic + i))
        # ic == 3 needs the last group's norm: emitted in the epilogue below

        # ---- prologue: just enough projection for the first group;
        # the first score pair is emitted before the x-chunk-0 V tiles so
        # exp#0 doesn't wait on them ----
        for g in qk_chunk_parts(wks, bkc, kt, 0, 0):
            g()
        for g in qk_chunk_parts(wqs, bqc, qt, 0, 0):
            g()
        sc_cur = scores_of[seq[0]](0)

        # ---- flat attention pipeline over all 8 groups ----
        for gi, (p, ic) in enumerate(seq):
            for j in range(TT):
                s = gi * TT + j
                pe = ptp.tile([128, 2 * NCH], f16, tag="pe", name="pe")
                nc.scalar.activation(pe[:], sc_cur[:], Exp, scale=0.125)
                pe_tiles[s] = pe
                if j + 1 < TT:
                    sc_cur = scores_of[(p, ic)](j + 1)
                elif gi + 1 < len(seq):
                    sc_cur = scores_of[seq[gi + 1]](0)  # no exp-stream break
                pvd = 1 if (gi == NG - 1 and j >= 12) else PVD
                add(s + pvd, pv_step(gi, j))
                if j == 0 and gi > 0:
                    ev, rc, mu, _ = norm_parts(gi - 1)
                    add(s + PVD - 1, ev)   # right after pv_step(gi-1, 15)
                    add(s + PVD, rc)
                    add(s + PVD + 1, mu)
                for th in work.get(s, ()):
                    th()
        for th in epilogue:                       # PV(7, 13) and PV(7, 15)
            th()
        ev, rc, mu, nst = norm_parts(NG - 1, tail=True)
        ev(); rc()
        # keep the PE busy (HAM warm) while the final norm chain runs so
        # the last out-proj tiles execute at 2.4GHz instead of 1.2
        warm2 = psq.tile([DH, DH], f32, tag="psq", name="warm2")
        for _ in range(16):
            nc.tensor.matmul(warm2[:], ones2d[:], ones2d[:],
                             start=True, stop=True)
        mu()
        # final 4 out-proj tiles, chunk-interleaved: the head-pair-1 rows
        # 64-127 come from tmp via split-K matmuls (no DMA on the tail);
        # PSUM evacuations alternate between the now-idle scalar engine
        # and the vector engine so two copies are always in flight
        obs = {t: osb.tile([128, D], f16, tag="ob", name="ob%d" % t)
               for t in (12, 14)}
        obs[13] = osb.tile([128, D], f16, tag="ob2", name="ob13")
        obs[15] = osb.tile([128, D], f16, tag="ob2", name="ob15")
        for c in range(D // NCH):
            for ti, t in enumerate((12, 13, 14, 15)):
                pp = psq.tile([128, NCH], f32, tag="psq", name="pp")
                nc.tensor.matmul(
                    pp[:], at[:, 0, ts(t, 128)], wos[:, 0, ts(c, NCH)],
                    start=True, stop=False,
                )
                nc.tensor.matmul(
                    pp[:], at[0:DH, 1, ts(t, 128)],
                    wos[0:DH, 1, ts(c, NCH)], start=False, stop=False,
                )
                nc.tensor.matmul(
                    pp[:], nst['tmp'][:, ts(t - 12, 128)],
                    wosl[:, 1, ts(c, NCH)], start=False, stop=True,
                )
                if (c + ti) % 2 == 0:
                    nc.scalar.copy(obs[t][:, ts(c, NCH)], pp[:])
                else:
                    nc.vector.tensor_copy(obs[t][:, ts(c, NCH)], pp[:])
        for t in (12, 13, 14, 15):
            nc.sync.dma_start(out[ts(t, 128), :], obs[t][:])

    nc.finalize()  # Bacc.compile(): wait legalization, reg alloc, act tables
    return nc


def _get_program():
    global _prog
    if _prog is None:
        _prog = _build()
    return _prog


def kernel(x, W_qkv, b_qkv, W_out, b_out):
    global LAST_RESULT
    from concourse.bass_utils import run_bass_kernel_spmd

    x = np.asarray(x, np.float32)
    W_qkv = np.asarray(W_qkv, np.float32)
    b_qkv = np.asarray(b_qkv, np.float32)
    W_out = np.asarray(W_out, np.float32)
    b_out = np.asarray(b_out, np.float32)

    nc = _get_program()

    in_maps = []
    for c in range(NCORES):
        b, g = divmod(c, GROUPS)
        sl = slice(g * F, (g + 1) * F)
        # interleave Wv/bv with [zero-weight, bias=1] columns at h*65+64
        wv_g = W_qkv[:, 2 * D:3 * D][:, sl]
        bv_g = b_qkv[2 * D:3 * D][sl]
        wv_i = np.zeros((D, VF), np.float16)
        bv_i = np.zeros((1, VF), np.float16)
        for h in range(HPC):
            wv_i[:, h * VW: h * VW + DH] = wv_g[:, h * DH:(h + 1) * DH]
            bv_i[0, h * VW: h * VW + DH] = bv_g[h * DH:(h + 1) * DH]
            bv_i[0, h * VW + DH] = 1.0
        in_maps.append({
            "xT": np.ascontiguousarray(x[b].T.astype(np.float16)),
            "wq": np.ascontiguousarray(W_qkv[:, 0 * D:1 * D][:, sl]).astype(np.float16),
            "wk": np.ascontiguousarray(W_qkv[:, 1 * D:2 * D][:, sl]).astype(np.float16),
            "wv": wv_i,
            "bq": np.ascontiguousarray(b_qkv[0 * D:1 * D][sl][:, None]),
            "bk": np.ascontiguousarray(b_qkv[1 * D:2 * D][sl][:, None]),
            "bv": bv_i,
            "wo": np.ascontiguousarray(W_out[sl, :]).astype(np.float16),
        })

    kw = {}
    if os.environ.get("KERNEL_TRACE") == "1":
        kw["trace"] = True
    res = run_bass_kernel_spmd(nc, in_maps, core_ids=list(range(NCORES)), **kw)
    LAST_RESULT = res

    out = np.empty((B, T, D), np.float32)
    for b in range(B):
        acc = res.results[GROUPS * b]["out"].astype(np.float32)
        for g in range(1, GROUPS):
            acc = acc + res.results[GROUPS * b + g]["out"]
        out[b] = acc + b_out
    return out


# revision 53
# speedup vs baseline: 1.0037x; 1.0037x over previous
"""Multi-head attention (B=2, T=2048, D=1024, H=16, Dh=64) on 8 TRN2 NeuronCores.

Sharding: core c = 4*b + g  ->  batch b in {0,1}, head-group g in {0..3}
(4 heads per core: data parallel on batch, tensor parallel on heads).
Each core computes, for its batch element and its 4 heads:

  Q.T/K.T = Wq/k_shard.T @ x.T + b      [256, 2048]  (head-dim on partitions)
  V'      = x @ Wv_interleaved + b      [2048, 260]  ([V_h | 1] per head)
  per head pair (2p, 2p+1), per 512-wide i-chunk:
    S.T   = K_h Q_h.T                   (two K=64 matmuls on disjoint PE
                                         row groups -> run concurrently)
    P.T   = exp(S.T / 8)                (no max-subtraction: |S|/8 <~ 6.6)
    acc   = [V_h | 1].T @ P.T           [65, 512]  row 64 = softmax denom
    attnT = acc[:64] * (1/acc[64])
  partial = attnT.T @ Wout_shard        [2048, 1024]

The partial sum over the 4 head groups plus b_out is done on the host
("all-reduce after out_proj"), as is the batch unshard.

The kernel is one flat software pipeline over the 8 attention groups.
The scalar engine's exp stream (128 x [128,1024] Exp ~ 147us) and the
PE matmul stream (~150us of issue time) are co-saturated; in steady
state the PE runs at 100% occupancy, so scheduling aims to keep both
engines from ever stalling each other:

  - input DMA uses one batched trigger per tensor (DMA triggers
    serialize at ~0.6us each on the sync queue) with x streamed in
    token-chunk-major order so the first scores issue ~13us in;
  - ~56 dummy matmuls warm the PE HAM clock gate (cold 1.2GHz ->
    2.4GHz needs ~3.4us of sustained activity) while the first DMAs
    land, and 16 more keep it warm across the final norm chain;
  - PV matmuls are emitted 3 exp-steps late (8 pe buffers) so a PV
    waiting on the shared PSUM accumulator never blocks a later score
    matmul at the head of the in-order PE queue;
  - softmax normalization first evacuates the accumulators to SBUF
    f16 (two [65,512] copies release the PSUM banks for the next
    group), then per head broadcasts the denominator row with a K=1
    matmul issued from partition 64 and takes a reciprocal of the
    broadcast tile; one [64,512] multiply per head writes attnT
    (head 2p+1 via tmp + SBUF->SBUF DMA, since DVE lanes cannot
    shift partitions);
  - V / K / Q projection chunks and out-projection tiles are spread
    across the exp steps as filler work sized to each group's slack;
  - the final group's out-projection reads tmp directly via split-K
    matmuls (no DMA wait on the tail), with PSUM evacuations
    alternating between the idle scalar engine and the vector engine;
  - output partials are written as f16 (the host sums them in f32).
"""

import os
from collections import defaultdict

import numpy as np

B, T, D = 2, 2048, 1024
H, DH = 16, 64
NCORES, GROUPS = 8, 4
HPC = H // GROUPS        # 4 heads per core
F = HPC * DH             # 256 features per core
FT = F // 128            # 2 feature tiles / head pairs
KTN = D // 128           # 8 contraction tiles
TT = T // 128            # 16 token tiles
NCH = 512                # matmul free-dim chunk
VW = DH + 1              # 65: V plus ones column
VF = HPC * VW            # 260: interleaved [V_h | 1] x 4 heads
NG = FT * (T // NCH)     # 8 attention groups
PVD = 3                  # PV emission delay in exp-steps

_prog = None
LAST_RESULT = None


def _build():
    from contextlib import ExitStack

    import concourse.mybir as mybir
    import concourse.tile as tile
    from concourse import bacc
    from concourse.bass import ts

    f32 = mybir.dt.float32
    f32r = mybir.dt.float32r
    f16 = mybir.dt.float16
    Exp = mybir.ActivationFunctionType.Exp

    nc = bacc.Bacc()
    xT = nc.dram_tensor("xT", [D, T], f16, kind="ExternalInput")
    wq = nc.dram_tensor("wq", [D, F], f16, kind="ExternalInput")
    wk = nc.dram_tensor("wk", [D, F], f16, kind="ExternalInput")
    # wv/bv come pre-interleaved from the host: column h*65+64 is a zero
    # weight column whose bias is 1.0, producing the [V_h | 1] layout that
    # supplies the softmax-denominator row of the PV matmul for free.
    wv = nc.dram_tensor("wv", [D, VF], f16, kind="ExternalInput")
    bq = nc.dram_tensor("bq", [F, 1], f32, kind="ExternalInput")
    bk = nc.dram_tensor("bk", [F, 1], f32, kind="ExternalInput")
    bv = nc.dram_tensor("bv", [1, VF], f16, kind="ExternalInput")
    wo = nc.dram_tensor("wo", [F, D], f16, kind="ExternalInput")
    # f16 partials: the host f32-sums 4 of them; quantization ~1e-4 << budget
    out = nc.dram_tensor("out", [T, D], f16, kind="ExternalOutput")

    with ExitStack() as ctx:
        tc = ctx.enter_context(tile.TileContext(nc))
        pers = ctx.enter_context(tc.tile_pool(name="pers", bufs=1))
        ptp = ctx.enter_context(tc.tile_pool(name="ptp", bufs=8))
        osb = ctx.enter_context(tc.tile_pool(name="osb", bufs=2))
        msc = ctx.enter_context(tc.tile_pool(name="msc", bufs=2))
        psq = ctx.enter_context(tc.tile_pool(name="psq", bufs=2, space="PSUM"))
        pss = ctx.enter_context(tc.tile_pool(name="pss", bufs=2, space="PSUM"))
        pso = ctx.enter_context(tc.tile_pool(name="pso", bufs=1, space="PSUM"))

        xt = pers.tile([128, KTN, T], f16, tag="xt")
        wqs = pers.tile([128, KTN, F], f16, tag="wqs")
        wks = pers.tile([128, KTN, F], f16, tag="wks")
        wvs = pers.tile([128, KTN, VF], f16, tag="wvs")
        bqc = pers.tile([128, FT, 1], f32, tag="bqc")
        bkc = pers.tile([128, FT, 1], f32, tag="bkc")
        bvr = pers.tile([1, VF], f16, tag="bvr")
        ones_f = pers.tile([1, 128], f32, tag="ones_f")
        ones16 = pers.tile([1, 128], f16, tag="ones16")
        ones2f = pers.tile([128, DH], f32, tag="ones2f")
        ones2d = pers.tile([128, DH], f16, tag="ones2d")
        wos = pers.tile([128, FT, D], f16, tag="wos")
        wosl = pers.tile([DH, FT, D], f16, tag="wosl")  # rows 64-127, base 0
        qt = pers.tile([128, FT, T], f16, tag="qt")
        kt = pers.tile([128, FT, T], f16, tag="kt")
        vs = pers.tile([128, TT, VF], f16, tag="vs")
        at = pers.tile([128, FT, T], f16, tag="at")

        # ISA memset can't target f16; memset f32 then copy-convert
        nc.vector.memset(ones_f[:], 1.0)
        nc.vector.tensor_copy(ones16[:], ones_f[:])
        nc.vector.memset(ones2f[:], 1.0)
        nc.vector.tensor_copy(ones2d[:], ones2f[:])

        # ~3.6us of dummy matmuls while the input DMA streams in: the PE
        # HAM clock gate needs ~3.4us of sustained activity to lift the
        # cold 4/8 throttle, so the real prologue projections run at
        # 2.4GHz instead of 1.2GHz
        warm = psq.tile([DH, DH], f32, tag="psq", name="warm")
        for _ in range(56):
            nc.tensor.matmul(warm[:], ones2d[:], ones2d[:],
                             start=True, stop=True)

        # ---- input DMA: one batched trigger per tensor/chunk (DMA trigger
        # instructions serialize at ~600ns each on the sync queue), ordered
        # for fastest pipeline start ----
        def kp(ap):          # [(k p) f] DRAM view -> [p k f]
            return ap.rearrange("(k p) f -> p k f", p=128)

        def xchunk(c):       # two half-K triggers -> two parallel queues
            h = KTN // 2
            nc.sync.dma_start(
                xt[:, 0:h, ts(c, NCH)],
                kp(xT[0:h * 128, ts(c, NCH)]))
            nc.sync.dma_start(
                xt[:, h:KTN, ts(c, NCH)],
                kp(xT[h * 128:D, ts(c, NCH)]))

        nc.sync.dma_start(wks[:, :, :], kp(wk[:, :]))
        xchunk(0)
        nc.sync.dma_start(wqs[:, :, :], kp(wq[:, :]))
        for ft in range(FT):
            nc.sync.dma_start(bkc[:, ft, :], bk[ts(ft, 128), :])
            nc.sync.dma_start(bqc[:, ft, :], bq[ts(ft, 128), :])
        nc.sync.dma_start(bvr[:], bv[:])
        nc.sync.dma_start(wvs[:, :, :], kp(wv[:, :]))
        xchunk(1)
        xchunk(2)
        xchunk(3)
        nc.sync.dma_start(wos[:, :, :], kp(wo[:, :]))  # out-proj weights last
        for ft in range(FT):
            nc.sync.dma_start(wosl[:, ft, :], wo[128 * ft + 64:128 * ft + 128, :])

        # ---- work units ----
        # qk chunks are split into two half-K thunks emitted one exp-step
        # apart so a projection never puts >1us of PE work between two
        # score matmuls in the in-order PE queue.
        def qk_chunk_parts(wsb, bcol, dst, ft, c):
            st = {}
            def go1():
                ps = psq.tile([128, NCH], f32, tag="psq", name="ps")
                st['ps'] = ps
                for k in range(KTN // 2):
                    nc.tensor.matmul(
                        ps[:],
                        wsb[:, k, ts(ft, 128)],
                        xt[:, k, ts(c, NCH)],
                        start=(k == 0), stop=False,
                    )
            def go2():
                ps = st['ps']
                for k in range(KTN // 2, KTN):
                    nc.tensor.matmul(
                        ps[:],
                        wsb[:, k, ts(ft, 128)],
                        xt[:, k, ts(c, NCH)],
                        start=False, stop=(k == KTN - 1),
                    )
                nc.vector.tensor_scalar_add(
                    dst[:, ft, ts(c, NCH)], ps[:], bcol[:, ft, :]
                )
            return go1, go2

        def v_tile(t):
            def go():
                pv = psq.tile([128, VF], f32, tag="psq", name="pv")
                for k in range(KTN):
                    nc.tensor.matmul(
                        pv[:], xt[:, k, ts(t, 128)], wvs[:, k, :],
                        start=(k == 0), stop=False,
                    )
                # bias via ones-row (also writes the denominator 1.0 cols)
                nc.tensor.matmul(
                    pv[:], ones16[:, 0:128], bvr[:], start=False, stop=True
                )
                nc.vector.tensor_copy(vs[:, t, :], pv[:])
            return go

        def outproj_tile(t):
            def go():
                ob = osb.tile([128, D], f16, tag="ob", name="ob")
                for c in range(D // NCH):
                    pp = psq.tile([128, NCH], f32, tag="psq", name="pp")
                    for ft in range(FT):
                        nc.tensor.matmul(
                            pp[:],
                            at[:, ft, ts(t, 128)],
                            wos[:, ft, ts(c, NCH)],
                            start=(ft == 0), stop=(ft == FT - 1),
                        )
                    nc.vector.tensor_copy(ob[:, ts(c, NCH)], pp[:])
                nc.sync.dma_start(out[ts(t, 128), :], ob[:])
            return go

        def make_scores(p, ic):
            def scores(j):
                # disjoint PE row groups (partitions 0-63 / 64-127): the two
                # K=64 matmuls execute concurrently
                sc = pss.tile([128, 2 * NCH], f32, tag="sc", name="sc")
                for hh in range(2):
                    nc.tensor.matmul(
                        sc[:, ts(hh, NCH)],
                        kt[hh * 64: hh * 64 + DH, p, ts(j, 128)],
                        qt[hh * 64: hh * 64 + DH, p, ts(ic, NCH)],
                        start=True, stop=True,
                    )
                return sc
            return scores

        seq = [(p, ic) for p in range(FT) for ic in range(T // NCH)]
        scores_of = {g: make_scores(*g) for g in seq}

        pe_tiles = {}      # global exp-step -> pe tile
        acc_of = {}        # gi -> (acc0, acc1)

        def pv_step(gi, j):
            p, _ = seq[gi]
            def go():
                if j == 0:
                    a0 = pso.tile([VW, NCH], f32, tag="acc0", name="acc0")
                    a1 = pso.tile([VW, NCH], f32, tag="acc1", name="acc1")
                    acc_of[gi] = (a0, a1)
                accs = acc_of[gi]
                pe = pe_tiles.pop(gi * TT + j)
                for hh in range(2):
                    nc.tensor.matmul(
                        accs[hh][:, :],
                        vs[:, j, (2 * p + hh) * VW: (2 * p + hh + 1) * VW],
                        pe[:, ts(hh, NCH)],
                        start=(j == 0), stop=(j == TT - 1),
                    )
            return go

        # softmax normalization for group gi, in three thunks:
        #   a) evacuate acc0/acc1 to SBUF f16 ([65,512] copies) -- this
        #      alone releases the accumulator PSUM banks for the next group
        #   b) per head: broadcast the denominator row across 64 partitions
        #      with a K=1 fp16 matmul issued from partition 64, then take
        #      the reciprocal of the broadcast PSUM tile
        #   c) attnT = num * (1/den): head 2p to at[0:64]; head 2p+1 via a
        #      [64,512] tmp + SBUF->SBUF DMA (DVE lanes can't shift
        #      partitions)
        def norm_parts(gi, tail=False):
            p, ic = seq[gi]
            st = {}
            def evac():
                acc0, acc1 = acc_of[gi]
                n0 = msc.tile([VW, NCH], f16, tag="n0", name="n0")
                n1 = msc.tile([VW, NCH], f16, tag="n1", name="n1")
                if tail:     # scalar engine is idle after the exp stream
                    nc.scalar.copy(n0[:], acc0[:])
                else:
                    nc.vector.tensor_copy(n0[:], acc0[:])
                nc.vector.tensor_copy(n1[:], acc1[:])
                st['n0'], st['n1'] = n0, n1
            def recip():
                for hh in range(2):
                    pbd = psq.tile([DH, NCH], f32, tag="psq", name="pbd")
                    nc.tensor.matmul(
                        pbd[:], ones2d[64:65, :],
                        st['n%d' % hh][DH:VW, :], start=True, stop=True,
                    )
                    rcb = msc.tile([DH, NCH], f32, tag="rcb%d" % hh,
                                   name="rcb")
                    nc.vector.reciprocal_approx_fast(rcb[:], pbd[:])
                    st['rcb%d' % hh] = rcb
            def muls():
                nc.vector.tensor_mul(
                    at[0:DH, p, ts(ic, NCH)], st['n0'][0:DH, :], st['rcb0'][:]
                )
                tmp = msc.tile([DH, NCH], f16, tag="tmp", name="tmp")
                nc.vector.tensor_mul(tmp[:], st['n1'][0:DH, :], st['rcb1'][:])
                if tail:
                    # final group: out-proj reads tmp directly (split-K
                    # matmuls) so no DMA round-trip sits on the tail
                    st['tmp'] = tmp
                else:
                    nc.sync.dma_start(at[64:128, p, ts(ic, NCH)], tmp[:])
            return evac, recip, muls, st

        # ---- static filler schedule (global exp-step -> thunks) ----
        GS = NG * TT
        work = defaultdict(list)
        epilogue = []
        def add(s, th):
            if s < GS:
                work[s].append(th)
            else:
                epilogue.append(th)

        def add_qk(s, *args):
            g1, g2 = qk_chunk_parts(*args)
            add(s, g1)
            add(s + 1, g2)

        for j in range(2, TT):                    # V proj just-in-time
            add(j - 2, v_tile(j))
        add_qk(0, wks, bkc, kt, 0, 1)             # K chunks for group 0
        add_qk(4, wks, bkc, kt, 0, 2)
        add_qk(8, wks, bkc, kt, 0, 3)
        add_qk(17, wks, bkc, kt, 1, 0)            # K chunks for groups 4-7,
        add_qk(21, wks, bkc, kt, 1, 1)            # spread over groups 1-3
        add_qk(33, wks, bkc, kt, 1, 2)
        add_qk(37, wks, bkc, kt, 1, 3)
        add_qk(12, wqs, bqc, qt, 0, 1)            # Q chunks, one group ahead
        add_qk(26, wqs, bqc, qt, 0, 2)
        add_qk(42, wqs, bqc, qt, 0, 3)
        add_qk(56, wqs, bqc, qt, 1, 0)
        add_qk(72, wqs, bqc, qt, 1, 1)
        add_qk(60, wqs, bqc, qt, 1, 2)            # keep the out-proj groups
        add_qk(76, wqs, bqc, qt, 1, 3)            # (5-7) free of projections
        for ic in range(3):                       # out-proj once both head
            for i, off in enumerate((5, 7, 9, 11)):    # pairs' attnT exist
                add((5 + ic) * TT + off, outproj_tile(4 * ic + i))
        # ic == 3 needs the last group's norm: emitted in the epilogue below

        # ---- prologue: just enough projection for the first group;
        # the first score pair is emitted before the x-chunk-0 V tiles so
        # exp#0 doesn't wait on them ----
        for g in qk_chunk_parts(wks, bkc, kt, 0, 0):
            g()
        for g in qk_chunk_parts(wqs, bqc, qt, 0, 0):
            g()
        sc_cur = scores_of[seq[0]](0)
        v_tile(0)()
        v_tile(1)()

        # ---- flat attention pipeline over all 8 groups ----
        for gi, (p, ic) in enumerate(seq):
            for j in range(TT):
                s = gi * TT + j
                pe = ptp.tile([128, 2 * NCH], f16, tag="pe", name="pe")
                nc.scalar.activation(pe[:], sc_cur[:], Exp, scale=0.125)
                pe_tiles[s] = pe
                if j + 1 < TT:
                    sc_cur = scores_of[(p, ic)](j + 1)
                elif gi + 1 < len(seq):
                    sc_cur = scores_of[seq[gi + 1]](0)  # no exp-stream break
                pvd = 1 if (gi == NG - 1 and j >= 12) else PVD
                add(s + pvd, pv_step(gi, j))
                if j == 0 and gi > 0:
                    ev, rc, mu, _ = norm_parts(gi - 1)
                    add(s + PVD - 1, ev)   # right after pv_step(gi-1, 15)
                    add(s + PVD, rc)
                    add(s + PVD + 1, mu)
                for th in work.get(s, ()):
                    th()
        for th in epilogue:                       # PV(7, 13) and PV(7, 15)
            th()
        ev, rc, mu, nst = norm_parts(NG - 1, tail=True)
        ev(); rc()
        # keep the PE busy (HAM warm) while the final norm chain runs so
        # the last out-proj tiles execute at 2.4GHz instead of 1.2
        warm2 = psq.tile([DH, DH], f32, tag="psq", name="warm2")
        for _ in range(16):
            nc.tensor.matmul(warm2[:], ones2d[:], ones2d[:],
                             start=True, stop=True)
        mu()
        # final 4 out-proj tiles, chunk-interleaved: the head-pair-1 rows
        # 64-127 come from tmp via split-K matmuls (no DMA on the tail);
        # PSUM evacuations alternate between the now-idle scalar engine
        # and the vector engine so two copies are always in flight
        obs = {t: osb.tile([128, D], f16, tag="ob", name="ob%d" % t)
               for t in (12, 14)}
        obs[13] = osb.tile([128, D], f16, tag="ob2", name="ob13")
        obs[15] = osb.tile([128, D], f16, tag="ob2", name="ob15")
        for c in range(D // NCH):
            for ti, t in enumerate((12, 13, 14, 15)):
                pp = psq.tile([128, NCH], f32, tag="psq", name="pp")
                nc.tensor.matmul(
                    pp[:], at[:, 0, ts(t, 128)], wos[:, 0, ts(c, NCH)],
                    start=True, stop=False,
                )
                nc.tensor.matmul(
                    pp[:], at[0:DH, 1, ts(t, 128)],
                    wos[0:DH, 1, ts(c, NCH)], start=False, stop=False,
                )
                nc.tensor.matmul(
                    pp[:], nst['tmp'][:, ts(t - 12, 128)],
                    wosl[:, 1, ts(c, NCH)], start=False, stop=True,
                )
                if (c + ti) % 2 == 0:
                    nc.scalar.copy(obs[t][:, ts(c, NCH)], pp[:])
                else:
                    nc.vector.tensor_copy(obs[t][:, ts(c, NCH)], pp[:])
        for t in (12, 13, 14, 15):
            nc.sync.dma_start(out[ts(t, 128), :], obs[t][:])

    nc.finalize()  # Bacc.compile(): wait legalization, reg alloc, act tables
    return nc


def _get_program():
    global _prog
    if _prog is None:
        _prog = _build()
    return _prog


def kernel(x, W_qkv, b_qkv, W_out, b_out):
    global LAST_RESULT
    from concourse.bass_utils import run_bass_kernel_spmd

    x = np.asarray(x, np.float32)
    W_qkv = np.asarray(W_qkv, np.float32)
    b_qkv = np.asarray(b_qkv, np.float32)
    W_out = np.asarray(W_out, np.float32)
    b_out = np.asarray(b_out, np.float32)

    nc = _get_program()

    in_maps = []
    for c in range(NCORES):
        b, g = divmod(c, GROUPS)
        sl = slice(g * F, (g + 1) * F)
        # interleave Wv/bv with [zero-weight, bias=1] columns at h*65+64
        wv_g = W_qkv[:, 2 * D:3 * D][:, sl]
        bv_g = b_qkv[2 * D:3 * D][sl]
        wv_i = np.zeros((D, VF), np.float16)
        bv_i = np.zeros((1, VF), np.float16)
        for h in range(HPC):
            wv_i[:, h * VW: h * VW + DH] = wv_g[:, h * DH:(h + 1) * DH]
            bv_i[0, h * VW: h * VW + DH] = bv_g[h * DH:(h + 1) * DH]
            bv_i[0, h * VW + DH] = 1.0
        in_maps.append({
            "xT": np.ascontiguousarray(x[b].T.astype(np.float16)),
            "wq": np.ascontiguousarray(W_qkv[:, 0 * D:1 * D][:, sl]).astype(np.float16),
            "wk": np.ascontiguousarray(W_qkv[:, 1 * D:2 * D][:, sl]).astype(np.float16),
            "wv": wv_i,
            "bq": np.ascontiguousarray(b_qkv[0 * D:1 * D][sl][:, None]),
            "bk": np.ascontiguousarray(b_qkv[1 * D:2 * D][sl][:, None]),
            "bv": bv_i,
            "wo": np.ascontiguousarray(W_out[sl, :]).astype(np.float16),
        })

    kw = {}
    if os.environ.get("KERNEL_TRACE") == "1":
        kw["trace"] = True
    res = run_bass_kernel_spmd(nc, in_maps, core_ids=list(range(NCORES)), **kw)
    LAST_RESULT = res

    out = np.empty((B, T, D), np.float32)
    for b in range(B):
        acc = res.results[GROUPS * b]["out"].astype(np.float32)
        for g in range(1, GROUPS):
            acc = acc + res.results[GROUPS * b + g]["out"]
        out[b] = acc + b_out
    return out
